# revision 39
# baseline (speedup 1.0000x reference)
"""Luong attention kernel for Trainium2 (Bass/Tile), batch-parallel over 8 NeuronCores.

Problem (per full input):
    enc_mask [64, 2048] bool, enc_out [64, 2048, 1024] f32, dec_hid [64, 1024] f32
    sims    = einsum('bsd,bd->bs', enc_out, dec_hid); masked -> -inf
    attn    = softmax(sims, axis=1)
    context = einsum('bs,bsd->bd', attn, enc_out)

Strategy: pure data parallelism -- batch dim 64 is split 8 ways (8 examples
per core).  Per core, enc_out (64 MB) is streamed through SBUF exactly once:
  * einsum1 (contract d): DVE tensor_tensor_reduce (fused mult+free-dim-reduce)
    against a broadcast copy of dec_hid, per 128-row chunk of s.
  * softmax: free-dim reduce + GPSIMD partition all-reduce for max/sum,
    ScalarE exp with fused accumulation.
  * einsum2 (contract s): TensorE matmuls, exp-weights as the [128,1]
    stationary operand, enc chunks as the moving operand, accumulated in PSUM.

s is laid out as s = p*CH + c (p = SBUF partition, c = chunk), which makes
every DMA fully contiguous per partition.
"""

from contextlib import ExitStack

import numpy as np

import concourse.bacc as bacc
import concourse.bass as bass
import concourse.tile as tile
from concourse import bass_isa, library_config, mybir
from concourse.bass_utils import run_bass_kernel_spmd

B, S, D = 64, 2048, 1024
N_CORES = 8
BPC = B // N_CORES  # examples per core
P = 128  # SBUF partitions

NEG_BIG = -1.0e30


def build_kernel_body(ctx: ExitStack, tc: "tile.TileContext", enc, msk, dec, out,
                      bpc: int, s: int, d: int, dma_chunks: int = 4,
                      n_gp_reduce: int = 2):
    nc = tc.nc
    ch = s // P                     # chunks of 128 s-values per example
    n_dma = max(1, ch // dma_chunks)
    dq = ch // n_dma                # chunks per DMA
    n_gp_reduce = min(n_gp_reduce, ch)
    # d split into <=512-wide segments (fp32 moving-operand / PSUM bank limit)
    d_segs = [(h, min(512, d - h)) for h in range(0, d, 512)]

    encp = ctx.enter_context(tc.tile_pool(name="encp", bufs=3))
    prodp = ctx.enter_context(tc.tile_pool(name="prodp", bufs=3))
    scrp = ctx.enter_context(tc.tile_pool(name="scrp", bufs=2))
    decrp = ctx.enter_context(tc.tile_pool(name="decrp", bufs=2))
    decbp = ctx.enter_context(tc.tile_pool(name="decbp", bufs=2))
    smallp = ctx.enter_context(tc.tile_pool(name="smallp", bufs=2))
    outp = ctx.enter_context(tc.tile_pool(name="outp", bufs=2))
    psum_b = ctx.enter_context(tc.tile_pool(name="psum_b", bufs=2, space="PSUM"))
    psum_c = ctx.enter_context(tc.tile_pool(name="psum_c", bufs=2, space="PSUM"))

    # GPSIMD custom ops (partition_all_reduce) live in a loadable library.
    nc.gpsimd.load_library(library_config.attnmlp)

    ones = smallp.tile([1, P], mybir.dt.float32, bufs=1)
    nc.vector.memset(ones, 1.0)

    # ---- masks for all examples in one shot: [128, bpc, ch] {0,1} -> -1e30
    mask_all = smallp.tile([P, bpc, ch], mybir.dt.uint8, tag="mask_all", bufs=1)
    nc.sync.dma_start(out=mask_all,
                      in_=msk.rearrange("b (p c) -> p b c", p=P))
    maskneg_all = smallp.tile([P, bpc, ch], mybir.dt.float32, tag="maskneg_all",
                              bufs=1)
    nc.vector.tensor_scalar_mul(maskneg_all, mask_all, NEG_BIG)

    # delayed epilogues: (psum ctx tile, 1/L tile, example index) emitted one
    # example late so the DVE's in-order stream never waits on TensorE
    pending = []

    def flush_epilogue():
        ctxps_, invl_, b_ = pending.pop(0)
        ctx_sb = outp.tile([1, d], mybir.dt.float32, tag="ctx_sb")
        nc.vector.tensor_scalar_mul(ctx_sb, ctxps_, invl_[0:1, :])
        nc.sync.dma_start(out=out[b_ : b_ + 1, :], in_=ctx_sb)

    for b in range(bpc):
        # ---- load dec_hid row and broadcast it to all 128 partitions (via PE)
        dec_row = decrp.tile([1, d], mybir.dt.float32, tag="dec_row")
        nc.sync.dma_start(out=dec_row, in_=dec[b : b + 1, :])
        dec_ps = psum_b.tile([P, d], mybir.dt.float32, tag="dec_ps")
        for h0, hw in d_segs:
            nc.tensor.matmul(dec_ps[:, h0 : h0 + hw], lhsT=ones,
                             rhs=dec_row[:, h0 : h0 + hw], start=True, stop=True)
        dec_b = decbp.tile([P, d], mybir.dt.float32, tag="dec_b")
        nc.vector.tensor_copy(dec_b, dec_ps)

        # ---- stream enc_out for this example; s = p*ch + c layout
        # einsum1: sims[p, c] = sum_d enc[s,d] * dec[d].
        # DVE multiplies (a quarter-example per instruction); ScalarE reduces
        # each chunk over the free dim via a Copy-activation whose fused
        # accumulator yields sims and whose main output downcasts prod to
        # bf16.  The bf16 prod replaces enc as einsum2's moving operand
        # (contract s): sum_s w*prod = dec .* context, fixed up on the host
        # by an elementwise divide.  This halves TensorE work (bf16 matmul
        # is single-pass, fp32 is double-pass) and lets enc tiles release
        # right after the multiply so the DMA stream never stalls.
        enc3 = enc[b].rearrange("(p c) d -> p c d", p=P)
        sims_raw = smallp.tile([P, ch], mybir.dt.float32, tag="sims_raw")
        prod_bf = scrp.tile([P, ch, d], mybir.dt.bfloat16, tag="prod_bf")
        dec_b_rep = bass.AP(
            tensor=dec_b.tensor,
            offset=dec_b.offset,
            ap=[dec_b.ap[0], [0, dq], dec_b.ap[1]],
        )
        for q in range(n_dma):
            enc_q = encp.tile([P, dq, d], mybir.dt.float32, tag="enc")
            nc.sync.dma_start(out=enc_q, in_=enc3[:, q * dq : (q + 1) * dq, :])
            prod = prodp.tile([P, dq, d], mybir.dt.float32, tag="prod")
            nc.vector.tensor_mul(prod, enc_q, dec_b_rep)
            for cc in range(dq):
                c = q * dq + cc
                if c < n_gp_reduce:
                    # GPSIMD path: reduce+downcast on the otherwise-idle
                    # Pool engine to relieve ScalarE
                    nc.gpsimd.tensor_scalar(
                        out=prod_bf[:, c, :],
                        in0=prod[:, cc, :],
                        scalar1=1.0,
                        scalar2=0.0,
                        op0=mybir.AluOpType.mult,
                        op1=mybir.AluOpType.add,
                        accum_out=sims_raw[:, c : c + 1],
                    )
                else:
                    nc.scalar.activation(prod_bf[:, c, :], prod[:, cc, :],
                                         mybir.ActivationFunctionType.Copy,
                                         accum_out=sims_raw[:, c : c + 1])
        sims = smallp.tile([P, ch], mybir.dt.float32, tag="sims")
        nc.vector.tensor_add(sims, sims_raw, maskneg_all[:, b, :])

        # ---- softmax pieces: global max, exp, sum
        maxcol = smallp.tile([P, 1], mybir.dt.float32, tag="maxcol")
        nc.vector.reduce_max(maxcol, sims, axis=mybir.AxisListType.X)
        maxall = smallp.tile([P, 1], mybir.dt.float32, tag="maxall")
        nc.gpsimd.partition_all_reduce(maxall, maxcol, channels=P,
                                       reduce_op=bass_isa.ReduceOp.max)
        negmax = smallp.tile([P, 1], mybir.dt.float32, tag="negmax")
        nc.vector.tensor_scalar_mul(negmax, maxall, -1.0)

        expw = smallp.tile([P, ch], mybir.dt.float32, tag="expw")
        expsum = smallp.tile([P, 1], mybir.dt.float32, tag="expsum")
        nc.scalar.activation(expw, sims, mybir.ActivationFunctionType.Exp,
                             bias=negmax, scale=1.0, accum_out=expsum)
        lsum = smallp.tile([P, 1], mybir.dt.float32, tag="lsum")
        nc.gpsimd.partition_all_reduce(lsum, expsum, channels=P,
                                       reduce_op=bass_isa.ReduceOp.add)
        invl = smallp.tile([P, 1], mybir.dt.float32, tag="invl")
        nc.vector.reciprocal(invl, lsum)

        # ---- einsum2 (over prod_bf): dec.*context = sum_{p,c} w * prod
        expw_bf = smallp.tile([P, ch], mybir.dt.bfloat16, tag="expw_bf")
        nc.vector.tensor_copy(expw_bf, expw)
        ctxps = psum_c.tile([1, d], mybir.dt.float32, tag="ctxps")
        for c in range(ch):
            for h0, hw in d_segs:
                nc.tensor.matmul(
                    ctxps[:, h0 : h0 + hw],
                    lhsT=expw_bf[:, c : c + 1],
                    rhs=prod_bf[:, c, h0 : h0 + hw],
                    start=(c == 0),
                    stop=(c == ch - 1),
                )

        # ---- scale by 1/sum(exp) and store (deferred one example)
        pending.append((ctxps, invl, b))
        if len(pending) > 1:
            flush_epilogue()

    while pending:
        flush_epilogue()


def build_nc(bpc: int = BPC, s: int = S, d: int = D, dma_chunks: int = 4,
             n_gp_reduce: int = 0):
    nc = bacc.Bacc("TRN2", target_bir_lowering=False, debug=False)
    enc = nc.dram_tensor("enc_out", [bpc, s, d], mybir.dt.float32,
                         kind="ExternalInput").ap()
    msk = nc.dram_tensor("enc_mask", [bpc, s], mybir.dt.uint8,
                         kind="ExternalInput").ap()
    dec = nc.dram_tensor("dec_hid", [bpc, d], mybir.dt.float32,
                         kind="ExternalInput").ap()
    out = nc.dram_tensor("context", [bpc, d], mybir.dt.float32,
                         kind="ExternalOutput").ap()
    with tile.TileContext(nc) as tc, ExitStack() as ctx:
        build_kernel_body(ctx, tc, enc, msk, dec, out, bpc, s, d, dma_chunks,
                          n_gp_reduce)
    nc.compile()
    return nc


_NC_CACHE = {}


def _get_nc():
    if "nc" not in _NC_CACHE:
        _NC_CACHE["nc"] = build_nc()
    return _NC_CACHE["nc"]


def run_sharded(enc_mask, enc_out, dec_hid, trace=False, **kw):
    """Shard over batch, run on 8 cores, return (full_output, BassKernelResults)."""
    nc = _get_nc()
    enc_mask = np.ascontiguousarray(enc_mask).astype(np.uint8)
    enc_out = np.ascontiguousarray(enc_out, dtype=np.float32)
    dec_hid = np.ascontiguousarray(dec_hid, dtype=np.float32)
    in_maps = [
        {
            "enc_mask": enc_mask[i * BPC : (i + 1) * BPC],
            "enc_out": enc_out[i * BPC : (i + 1) * BPC],
            "dec_hid": dec_hid[i * BPC : (i + 1) * BPC],
        }
        for i in range(N_CORES)
    ]
    res = run_bass_kernel_spmd(nc, in_maps, core_ids=list(range(N_CORES)),
                               trace=trace, **kw)
    full = np.concatenate([r["context"] for r in res.results], axis=0)
    # The device computes sum_s w[s] * (enc[s,:]*dec) = dec .* context;
    # undo the dec factor here.
    full = full / dec_hid
    return full, res


def kernel(enc_mask, enc_out, dec_hid):
    full, _ = run_sharded(enc_mask, enc_out, dec_hid)
    return full.astype(np.float32)
